# revision 27
# baseline (speedup 1.0000x reference)
"""Trainium2 Bass kernel for CrossAttention.

Problem (full shapes):
    query [16, 2048, 512], key [16, 2048, 256], value [16, 2048, 256]
    Wq [512,256] bq [256], Wk [256,256] bk [256], Wv [256,256] bv [256],
    Wo [256,256] bo [256]
    out = softmax((query@Wq+bq) @ (key@Wk+bk)^T / 16) @ (value@Wv+bv) @ Wo + bo

Strategy:
  - Data-parallel over batch: 8 cores x 2 batches each. Full weights on
    every core, no collectives.
  - Activations/weights cast to bf16 on host; all matmuls bf16 with fp32
    PSUM accumulation.
  - v is NEVER projected or transposed on device: since softmax rows sum
    to 1,  attn @ (v@Wv+bv) @ Wo + bo  ==  (attn@v) @ (Wv@Wo) + (bv@Wo+bo),
    and attn@v wants v with s on partitions -- its natural layout.  Wvo
    and the folded bias are precomputed on host in fp32.
  - q and k are DMA-xbar-transposed in 512-row strips.  HWDGE descriptor
    generation is a serial ~1.26us-per-strip resource, and transpose
    DMAs serialize pairwise against ANY other DRAM->SBUF DMA even across
    the two HWDGE rings (xbar/S2M hazard guard), so ALL input loads ride
    the sync ring, interleaved in consumption order (k strips / first q
    strip first).  Output stores (SBUF->DRAM, no S2M hazard) ride the
    ACT ring so batch-1 prefetch is never FIFO-blocked behind them.
  - ~40 warm-up matmuls on a junk tile at t=0 lift the PE HAM clock gate
    to 8/8 before the first real matmul arrives.
      KT[256,2048] = Wk^T @ kT (+bk), QT likewise (+bq); QT is projected
      strip-by-strip, interleaved between query blocks, in DMA order.
  - per 512-wide query block (kc-loop software-pipelined two deep so the
    ACT exp latency never stalls the PE):
      S^T[k,q] accumulated over 2 h-chunks; E = exp(S^T/16) (ACT)
      attT[d,q] += v[kc]-slices @ E      (PSUM accum over 16 k-chunks)
      softmax denominator without touching the PE stream: a progressive
      in-place pairwise tree on DVE sums the 16 E tiles (bf16), then 4
      tiny N=1 matmuls A_chunk^T @ ones give dT[q] in partition
      orientation -> DVE reciprocal.
      out_unscaled[q,v] = attT^T @ Wvo   (division commutes past Wvo)
      out[q,v] = out_unscaled * (1/d)[q] + bo_eff   (one DVE op)
    The whole post-kc-loop tail (dT matmuls, out-proj, scaling, store)
    is EMITTED inside the NEXT query block's kc loop, so the in-order PE
    queue never stalls waiting for the DVE tree.
  - softmax skips max-subtraction: scores here are ~N(0, 0.33), exp is
    safe in fp32 and matches the reference to ~1e-7.
"""

import functools
import os
import sys
from contextlib import ExitStack

import numpy as np

sys.path.insert(0, "/opt/trn_rl_repo")

import ml_dtypes  # noqa: E402

import concourse.bass as bass  # noqa: E402
import concourse.mybir as mybir  # noqa: E402
from concourse import bacc, tile  # noqa: E402
from concourse.bass_utils import run_bass_kernel_spmd  # noqa: E402

P = 128
N_CORES = 8
B, S, QD, KD, VD, HD = 16, 2048, 512, 256, 256, 256
B_LOC = B // N_CORES  # batches per core
QB = 512              # query block width
NQB = S // QB         # query blocks per batch
KC = S // P           # key chunks per batch
QC = QD // P          # qd chunks
HC = HD // P          # h chunks
SCALE = 1.0 / np.sqrt(HD)

BF = mybir.dt.bfloat16
F32 = mybir.dt.float32
AF = mybir.ActivationFunctionType
ALU = mybir.AluOpType

# wpack column offsets (host packs [Wk | Wq | Wvo], each (c p) h -> p (c h))
O_WK = 0
O_WQ = O_WK + HC * HD
O_WVO = O_WQ + QC * HD
WCOLS = O_WVO + HC * VD


def build_nc() -> bass.Bass:
    nc = bacc.Bacc("TRN2", target_bir_lowering=False, debug=False)

    query = nc.declare_dram_parameter("query", [B_LOC, S, QD], BF, isOutput=False)
    key = nc.declare_dram_parameter("key", [B_LOC, S, KD], BF, isOutput=False)
    value = nc.declare_dram_parameter("value", [B_LOC, S, VD], BF, isOutput=False)
    wpack = nc.declare_dram_parameter("wpack", [P, WCOLS], BF, isOutput=False)
    # bpack[p, :] = [bq2 (HC) | bk2 (HC) | bo_bc (VD)]
    bpack = nc.declare_dram_parameter("bpack", [P, 2 * HC + VD], F32,
                                      isOutput=False)
    # out in bf16: halves store traffic; ~1e-3 extra rel err, well under
    # the 2e-2 budget (host casts back to f32)
    out = nc.declare_dram_parameter("out", [B_LOC, S, VD], BF, isOutput=True)

    with tile.TileContext(nc) as tc, ExitStack() as ctx:
        const = ctx.enter_context(tc.tile_pool(name="const", bufs=1))
        pT = ctx.enter_context(tc.tile_pool(name="pT", bufs=2))
        pProj = ctx.enter_context(tc.tile_pool(name="pProj", bufs=2))
        pE = ctx.enter_context(tc.tile_pool(name="pE", bufs=4))
        pAtt = ctx.enter_context(tc.tile_pool(name="pAtt", bufs=4))
        pSmall = ctx.enter_context(tc.tile_pool(name="pSmall", bufs=4))
        pOut = ctx.enter_context(tc.tile_pool(name="pOut", bufs=8))
        ps_proj = ctx.enter_context(tc.tile_pool(name="ps_proj", bufs=2, space="PSUM"))
        ps_st = ctx.enter_context(tc.tile_pool(name="ps_st", bufs=3, space="PSUM"))
        ps_att = ctx.enter_context(tc.tile_pool(name="ps_att", bufs=2, space="PSUM"))
        ps_d = ctx.enter_context(tc.tile_pool(name="ps_d", bufs=1, space="PSUM"))

        wpack_sb = const.tile([P, WCOLS], BF)
        bpack_sb = const.tile([P, 2 * HC + VD], F32)

        # ---- input loads ----
        # k/q: DMA xbar transpose in 512-row strips; v: plain chunk loads.
        def load_k(b, eng):
            kT = pT.tile([P, KD // P, S], BF, tag="kT", name=f"kT{b}")
            for sc in range(S // QB):
                for c in range(KD // P):
                    eng.dma_start(
                        kT[:, c, sc * QB:(sc + 1) * QB],
                        key[b, sc * QB:(sc + 1) * QB, c * P:(c + 1) * P],
                        transpose=True,
                    )
            return kT

        def load_v_chunk(b, v_nat, sck, eng):
            # per-chunk loads keep the DRAM read fully contiguous (64KB)
            # so the v transfers never slow the transpose pipeline drain
            eng.dma_start(
                v_nat[:, sck, :],
                value[b, sck * P:(sck + 1) * P, :],
            )

        def load_q_strip(b, qT, sc, eng):
            for c in range(QC):
                eng.dma_start(
                    qT[:, c, sc * QB:(sc + 1) * QB],
                    query[b, sc * QB:(sc + 1) * QB, c * P:(c + 1) * P],
                    transpose=True,
                )

        # warm-up: ~40 junk matmuls keep the PE HAM busy from t~0.4us so
        # the clock gate is at 8/8 when the first projection data lands
        w_warm = const.tile([P, P], BF)
        nc.vector.memset(w_warm[:], 0.0)
        ps_warm = ps_d.tile([P, P], F32, tag="d", name="warm")
        for _ in range(40):
            nc.tensor.matmul(ps_warm[:], lhsT=w_warm[:], rhs=w_warm[:],
                             start=True, stop=True)

        # Ring discipline: every plain<->transpose switch on a ring costs
        # a ~5-7us pairwise-serialization drain.  So ALL plain loads
        # (biases, weights, both batches' v) go first, then ALL
        # transposes in consumption order — exactly one switch boundary.
        nc.sync.dma_start(wpack_sb[:, O_WK:O_WQ], wpack[:, O_WK:O_WQ])
        nc.sync.dma_start(wpack_sb[:, O_WQ:O_WVO], wpack[:, O_WQ:O_WVO])
        nc.sync.dma_start(wpack_sb[:, O_WVO:], wpack[:, O_WVO:])
        v_tiles = []
        for b in range(B_LOC):
            v_b = pT.tile([P, KC, VD], BF, tag="v", name=f"v{b}")
            for sck in range(KC):
                load_v_chunk(b, v_b, sck, nc.sync)
            v_tiles.append(v_b)
        # bpack last: the plain->transpose drain waits for the last
        # plain's completion, so make that one tiny
        nc.sync.dma_start(bpack_sb[:], bpack[:, :])
        # batch-0 transposes: k strips 0-1, q strip 0, k strips 2-3,
        # q strips 1-3 (matches the order compute consumes them)
        kT0 = pT.tile([P, KD // P, S], BF, tag="kT", name="kT0")
        for sc in range(2):
            for c in range(KD // P):
                nc.sync.dma_start(
                    kT0[:, c, sc * QB:(sc + 1) * QB],
                    key[0, sc * QB:(sc + 1) * QB, c * P:(c + 1) * P],
                    transpose=True,
                )
        qT0 = pT.tile([P, QC, S], BF, tag="qT", name="qT0")
        load_q_strip(0, qT0, 0, nc.sync)
        for sc in range(2, S // QB):
            for c in range(KD // P):
                nc.sync.dma_start(
                    kT0[:, c, sc * QB:(sc + 1) * QB],
                    key[0, sc * QB:(sc + 1) * QB, c * P:(c + 1) * P],
                    transpose=True,
                )
        for sc in range(1, S // QB):
            load_q_strip(0, qT0, sc, nc.sync)
        loaded0 = (kT0, v_tiles[0], qT0)

        wk_sb = wpack_sb[:, O_WK:O_WQ].rearrange("p (c h) -> p c h", c=HC)
        wq_sb = wpack_sb[:, O_WQ:O_WVO].rearrange("p (c h) -> p c h", c=QC)
        wvo_sb = wpack_sb[:, O_WVO:].rearrange("p (c h) -> p c h", c=HC)
        bq_sb = bpack_sb[:, 0:HC]
        bk_sb = bpack_sb[:, HC:2 * HC]
        bo_sb = bpack_sb[:, 2 * HC:]

        ones1 = const.tile([P, 1], BF)  # rhs for the denominator matmuls
        nc.vector.memset(ones1[:], 1.0)

        # deferred tail emission: tails[(b, qb)] is emitted partway into
        # the NEXT query block's kc loop (or at batch boundary / end)
        pending_tail = [None]
        stores = []

        def flush_tail():
            if pending_tail[0] is not None:
                pending_tail[0]()
                pending_tail[0] = None

        for b in range(B_LOC):
            if b == 0:
                kT, v_nat, qT = loaded0
            else:
                kT = load_k(b, nc.sync)
                v_nat = v_tiles[b]
                qT = pT.tile([P, QC, S], BF, tag="qT", name=f"qT{b}")
                for sc in range(S // QB):
                    load_q_strip(b, qT, sc, nc.sync)

            # ---- K projection, strip-major (consumed in DMA order) ----
            # KT[h,s] = Wk^T @ kT + bk (ACT bias-add, bf16 out)
            KT = pProj.tile([P, HC, S], BF, tag="KT")

            def k_proj_strip(sc, b=b, kT=kT, KT=KT):
                for hc in range(HC):
                    ps = ps_proj.tile([P, QB], F32, tag="proj", name=f"pk{b}{hc}{sc}")
                    for c in range(KD // P):
                        nc.tensor.matmul(
                            ps[:],
                            lhsT=wk_sb[:, c, hc * P:(hc + 1) * P],
                            rhs=kT[:, c, sc * QB:(sc + 1) * QB],
                            start=(c == 0),
                            stop=(c == KD // P - 1),
                        )
                    nc.scalar.activation(
                        KT[:, hc, sc * QB:(sc + 1) * QB], ps[:],
                        AF.Identity, bias=bk_sb[:, hc:hc + 1],
                    )

            # batch 0 is DMA-bound at the head: emit only K strips 0-1
            # before the first attention block; strips 2-3 are emitted
            # inside qb0's kc loop (they are only needed from kc=8)
            n_kp_head = 2 if b == 0 else S // QB
            for sc in range(n_kp_head):
                k_proj_strip(sc)

            # ---- Q projection, one strip at a time: strip qb is emitted
            # just before the attention block that consumes it ----
            QT = pProj.tile([P, HC, S], BF, tag="QT")

            def q_proj_strip(sc, b=b, qT=qT, QT=QT):
                for hc in range(HC):
                    ps = ps_proj.tile([P, QB], F32, tag="proj", name=f"pq{b}{hc}{sc}")
                    for c in range(QC):
                        nc.tensor.matmul(
                            ps[:],
                            lhsT=wq_sb[:, c, hc * P:(hc + 1) * P],
                            rhs=qT[:, c, sc * QB:(sc + 1) * QB],
                            start=(c == 0),
                            stop=(c == QC - 1),
                        )
                    nc.scalar.activation(
                        QT[:, hc, sc * QB:(sc + 1) * QB], ps[:],
                        AF.Identity, bias=bq_sb[:, hc:hc + 1],
                    )

            q_proj_strip(0)

            # ---- attention, one 512-wide query block at a time ----
            for qb in range(NQB):
                def emit_st(kc, b=b, qb=qb, KT=KT, QT=QT):
                    st = ps_st.tile([P, QB], F32, tag="st", name=f"st{b}_{qb}_{kc}")
                    for hc in range(HC):
                        nc.tensor.matmul(
                            st[:],
                            lhsT=KT[:, hc, kc * P:(kc + 1) * P],
                            rhs=QT[:, hc, qb * QB:(qb + 1) * QB],
                            start=(hc == 0),
                            stop=(hc == HC - 1),
                        )
                    return st

                # softmax denominator: linear running sum of the E tiles
                # on DVE (bf16, error ~1e-3 on d) — one add per kc, so
                # the post-loop chain is a single add
                acc = pSmall.tile([P, QB], BF, tag="acc", name=f"acc{b}_{qb}")
                att_ps = [
                    ps_att.tile([P, QB], F32, tag="att", name=f"att{b}_{qb}_{h}")
                    for h in range(HC)
                ]

                # software pipeline: keep two S^T tiles in flight so the
                # exp latency on ACT never blocks the PE matmul stream.
                st_tiles = [emit_st(0), emit_st(1)]
                e_tiles = []
                for kc in range(KC):
                    e_sb = pE.tile([P, QB], BF, tag="e", name=f"e{b}_{qb}_{kc}")
                    nc.scalar.activation(e_sb[:], st_tiles[kc][:],
                                         AF.Exp, scale=SCALE)
                    e_tiles.append(e_sb)
                    if kc + 2 < KC:
                        st_tiles.append(emit_st(kc + 2))
                    for hc in range(HC):
                        nc.tensor.matmul(
                            att_ps[hc][:],
                            lhsT=v_nat[:, kc, hc * P:(hc + 1) * P],
                            rhs=e_sb[:],
                            start=(kc == 0),
                            stop=(kc == KC - 1),
                        )
                    if kc == 1:
                        nc.vector.tensor_tensor(
                            acc[:], e_tiles[0][:], e_tiles[1][:], ALU.add)
                    elif kc > 1:
                        nc.vector.tensor_tensor(
                            acc[:], acc[:], e_sb[:], ALU.add)
                    if kc == 3:
                        flush_tail()
                    if kc == 4 and qb == 0:
                        for sc in range(n_kp_head, S // QB):
                            k_proj_strip(sc)

                # unnormalized attT -> SBUF (bf16) so the PSUM att banks
                # free up for the next block; division deferred past Wvo
                att_sb = [
                    pAtt.tile([P, QB], BF, tag="att_sb", name=f"attsb{b}_{qb}_{h}")
                    for h in range(HC)
                ]
                for hc in range(HC):
                    nc.vector.tensor_copy(att_sb[hc][:], att_ps[hc][:])

                def tail(b=b, qb=qb, acc=acc, att_sb=att_sb):
                    # dT[q] = sum_p acc[p, q] via 4 tiny N=1 matmuls ->
                    # 1/d with q already on partitions (no PE transposes)
                    dT_ps = ps_d.tile([P, QB // P], F32, tag="d",
                                      name=f"dT{b}_{qb}")
                    for qs in range(QB // P):
                        nc.tensor.matmul(
                            dT_ps[:, qs:qs + 1],
                            lhsT=acc[:, qs * P:(qs + 1) * P],
                            rhs=ones1[:],
                            start=True,
                            stop=True,
                        )
                    rT_sb = pSmall.tile([P, QB // P], F32, tag="rT",
                                        name=f"rT{b}_{qb}")
                    nc.vector.reciprocal(rT_sb[:], dT_ps[:])

                    # out[q, v] = (attT^T @ Wvo) * (1/d)[q] + bo_eff
                    o_all = pOut.tile([P, QB // P, VD], BF, tag="o",
                                      name=f"o{b}_{qb}")
                    for qs in range(QB // P):
                        ops = ps_proj.tile([P, VD], F32, tag="proj",
                                           name=f"po{b}_{qb}_{qs}")
                        for hc in range(HC):
                            nc.tensor.matmul(
                                ops[:],
                                lhsT=att_sb[hc][:, qs * P:(qs + 1) * P],
                                rhs=wvo_sb[:, hc, :],
                                start=(hc == 0),
                                stop=(hc == HC - 1),
                            )
                        nc.vector.scalar_tensor_tensor(
                            o_all[:, qs, :], ops[:], rT_sb[:, qs:qs + 1],
                            bo_sb[:], op0=ALU.mult, op1=ALU.add,
                        )
                    # store doorbells are deferred to the very end of the
                    # sync ring (behind all input transposes) so they
                    # never pairwise-serialize against in-flight loads
                    # or block a compute engine's sequencer
                    r0 = qb * QB
                    stores.append((
                        out[b, r0:r0 + QB, :].rearrange("(qs p) v -> p qs v",
                                                        p=P),
                        o_all,
                    ))

                pending_tail[0] = tail
                if qb + 1 < NQB:
                    q_proj_strip(qb + 1)

        flush_tail()
        for dram_ap, o_tile in stores:
            nc.sync.dma_start(dram_ap, o_tile[:])

    nc.finalize()
    return nc


@functools.cache
def _cached_nc() -> bass.Bass:
    return build_nc()


def _prep_in_maps(inputs: dict) -> list[dict]:
    bf16 = ml_dtypes.bfloat16
    q = np.ascontiguousarray(np.asarray(inputs["query"])).astype(bf16)
    k = np.ascontiguousarray(np.asarray(inputs["key"])).astype(bf16)
    v = np.ascontiguousarray(np.asarray(inputs["value"])).astype(bf16)
    bq = np.asarray(inputs["bq"], dtype=np.float32)
    bk = np.asarray(inputs["bk"], dtype=np.float32)
    bv = np.asarray(inputs["bv"], dtype=np.float32)
    bo = np.asarray(inputs["bo"], dtype=np.float32)
    Wv32 = np.asarray(inputs["Wv"], dtype=np.float32)
    Wo32 = np.asarray(inputs["Wo"], dtype=np.float32)

    # [128, c*h] per weight: rearrange (c p) h -> p (c h)
    def wprep(w, nchunk):
        w = np.asarray(w).astype(bf16)
        return w.reshape(nchunk, P, w.shape[1]).transpose(1, 0, 2).reshape(P, -1)

    wvo = (Wv32 @ Wo32).astype(np.float32)  # fold Wv into Wo (host, fp32)
    wpack = np.ascontiguousarray(np.concatenate(
        [wprep(inputs["Wk"], HC), wprep(inputs["Wq"], QC),
         wprep(wvo, HC)], axis=1))

    bq2 = bq.reshape(HC, P).T                                # [128, HC]
    bk2 = bk.reshape(HC, P).T
    bo_eff = (bv @ Wo32 + bo).astype(np.float32)             # fold bv
    bo_bc = np.broadcast_to(bo_eff, (P, VD))
    bpack = np.ascontiguousarray(
        np.concatenate([bq2, bk2, bo_bc], axis=1).astype(np.float32))

    in_maps = []
    for c in range(N_CORES):
        sl = slice(c * B_LOC, (c + 1) * B_LOC)
        in_maps.append({
            "query": np.ascontiguousarray(q[sl]),
            "key": np.ascontiguousarray(k[sl]),
            "value": np.ascontiguousarray(v[sl]),
            "wpack": wpack, "bpack": bpack,
        })
    return in_maps


def run(inputs: dict, **run_kwargs):
    """Run on 8 cores; returns (output [16,2048,256] f32, BassKernelResults)."""
    nc = _cached_nc()
    in_maps = _prep_in_maps(inputs)
    try:
        res = run_bass_kernel_spmd(nc, in_maps, core_ids=list(range(N_CORES)),
                                   **run_kwargs)
    except Exception:
        # transient device hiccups (e.g. NRT_EXEC_UNIT_UNRECOVERABLE after a
        # previous run) usually clear on retry
        import time
        time.sleep(10)
        res = run_bass_kernel_spmd(nc, in_maps, core_ids=list(range(N_CORES)),
                                   **run_kwargs)
    out = np.concatenate([res.results[c]["out"] for c in range(N_CORES)], axis=0)
    return out.astype(np.float32), res


def kernel(**inputs) -> np.ndarray:
    out, _ = run(inputs)
    return out
